# revision 31
# baseline (speedup 1.0000x reference)
"""Multi-head self-attention Trainium2 kernel (B=4, T=2048, D=512, H=8, HD=64).

Sharding: 8 cores = 4 batches x 2 head-groups (4 heads each). Each core:
  - DMA-XBAR-transposes bf16 x into SBUF (no PE transposes),
  - projects q,k (transposed layout) and v (natural layout, with 64 ones
    columns per head so the softmax denominator comes out of the PV matmul
    pre-broadcast across 64 partitions),
  - causal flash attention in transposed space, all matmuls bf16 except the
    PV step on i-tiles 1-3 which runs fp8 DoubleRow (two 128-key chunks
    contracted per pass at 0.5 cycles/row; v+ones in fp8e4, probs in fp8e5
    whose wide exponent range avoids inf/subnormal cliffs). i-tile 0 (rows
    with few valid keys, where fp8 noise does not average out) stays bf16.
  - exp on ACT with scale=1/8 and bias=-2 folded in (bias cancels in the
    softmax normalization; keeps probs inside fp8e5 range),
  - normalizes with reciprocal_approx_fast + tensor_mul,
  - output projection for its 256 contraction rows -> partial [2048, 512].
Host sums the 2 partials per batch (the TP all-reduce of the hint).

Scheduling: the whole kernel is emitted as one software-pipelined stream.
Attention processes two heads at once (ping-pong) so the PE computes one
head's S scores while the ACT engine exponentiates the other's, and
projection / output-projection matmul groups are injected as filler work
into the remaining exp-wait slots to keep the PE dense (sustains the 2.4GHz
p-state). PSUM: one shared 3-deep ring of 2-bank tiles (S scores + all
projections) + 2 PV accumulators = 8 banks.
"""

import sys

sys.path.insert(0, "/opt/trn_rl_repo")

import numpy as np
import ml_dtypes

import concourse.bass as bass
import concourse.tile as tile
from concourse import bacc, masks, mybir
from concourse.bass_utils import run_bass_kernel_spmd

f32 = mybir.dt.float32
bf16 = mybir.dt.bfloat16
f8 = mybir.dt.float8e4
f8p = mybir.dt.float8e5
u32 = mybir.dt.uint32

B, T, D, H, HD = 4, 2048, 512, 8, 64
NCORES = 8
SCALE = 1.0 / np.sqrt(HD)  # 0.125
EXP_BIAS = -2.0
NEG = -1.0e30
EXPF = mybir.ActivationFunctionType.Exp
DR = mybir.MatmulPerfMode.DoubleRow

_BUILT = None
DEBUG = False


def _build():
    nc = bacc.Bacc("TRN2", target_bir_lowering=False, debug=False)

    x_d = nc.dram_tensor("x", [T, D], bf16, kind="ExternalInput")
    wqk_d = nc.dram_tensor("wqk", [128, 4, 512], bf16, kind="ExternalInput")
    wv_d = nc.dram_tensor("wv", [128, 4, 256], bf16, kind="ExternalInput")
    wo_d = nc.dram_tensor("wo", [128, 2, 512], bf16, kind="ExternalInput")
    out_d = nc.dram_tensor("out", [T, D], f32, kind="ExternalOutput")
    dbg = {}
    if DEBUG:
        dbg["xT"] = nc.dram_tensor("dbg_xT", [128, 4, T], bf16, kind="ExternalOutput")
        dbg["qkT"] = nc.dram_tensor("dbg_qkT", [128, 4, T], bf16, kind="ExternalOutput")
        dbg["vaug"] = nc.dram_tensor(
            "dbg_vaug", [128, 16, 4, 128], f8, kind="ExternalOutput"
        )
        dbg["yTn"] = nc.dram_tensor("dbg_yTn", [128, 2, T], bf16, kind="ExternalOutput")

    with tile.TileContext(nc) as tc:
        with (
            tc.tile_pool(name="big", bufs=1) as big,
            tc.tile_pool(name="pp", bufs=5) as ppool,
            tc.tile_pool(name="rp", bufs=3) as rpool,
            tc.tile_pool(name="op", bufs=3) as opool,
            tc.tile_pool(name="ps2", bufs=3, space=bass.MemorySpace.PSUM) as psS2,
            tc.tile_pool(name="pv", bufs=2, space=bass.MemorySpace.PSUM) as psPV,
        ):
            # ---- persistent sbuf tensors ----
            xT = big.tile([128, 4, T], bf16)     # [d_part, d_chunk, t]
            qkT = big.tile([128, 4, T], bf16)    # ct: 0=q(h0,h1) 1=q(h2,h3) 2=k(h0,h1) 3=k(h2,h3)
            vaug8 = big.tile([128, 16, 4, 128], f8)   # [t_part, t_tile, head, 64 v + 64 ones]
            vaug16 = big.tile([128, 4, 4, 128], bf16)  # chunks 0-3 only (i-tile 0)
            yTn = big.tile([128, 2, T], bf16)    # [c_part, c_chunk, t]
            wqk_s = big.tile([128, 4, 512], bf16)
            wv_s = big.tile([128, 4, 256], bf16)
            wo_s = big.tile([128, 2, 512], bf16)

            # ---- constants ----
            up_f = big.tile([128, 128], f32)
            masks.make_causal_mask(nc, up_f[:], mask_val=NEG)  # [k,m]=NEG where m>k
            uprime = big.tile([128, 128], bf16)
            nc.vector.tensor_scalar_mul(uprime[:], up_f[:], 1.0)
            eid_f = big.tile([128, 512], f32)
            nc.vector.memset(eid_f[:], 0.0)
            masks.make_identity(nc, eid_f[:, 0:128], nomemset=True)
            e512 = big.tile([128, 512], bf16)
            nc.vector.tensor_scalar_mul(e512[:], eid_f[:], 1.0)

            # ones columns: fp8e4(1.0)=0x38, bf16(1.0)=0x3F80
            nc.vector.memset(vaug8[:, :, :, 64:128].bitcast(u32), 0x38383838)
            nc.vector.memset(vaug16[:, :, :, 64:128].bitcast(u32), 0x3F803F80)

            ebias = big.tile([128, 1], f32)
            nc.vector.memset(ebias[:], EXP_BIAS)

            # ---- weight loads (gpsimd SWDGE queue; hwdge queues do x) ----
            nc.scalar.dma_start(wqk_s[:], wqk_d.ap())
            nc.scalar.dma_start(wv_s[:], wv_d.ap())
            nc.gpsimd.dma_start(wo_s[:], wo_d.ap())

            # ---- phase 1: XBAR dma transposes of x into xT, t4-chunked,
            # split across both HWDGE queues (sync + scalar) ----
            # NB: XBAR transposes must all stay on ONE queue — two queues
            # driving the XBAR concurrently corrupt each other's tiles.
            for t4 in range(4):
                for cc in range(4):
                    eng = nc.sync
                    eng.dma_start_transpose(
                        xT[:, cc, t4 * 512:(t4 + 1) * 512],
                        x_d.ap()[t4 * 512:(t4 + 1) * 512, cc * 128:(cc + 1) * 128],
                    )

            # ---- emission helpers (each is a self-contained PE work unit) --
            def emit_qk(ct, t4):
                pst = psS2.tile([128, 1024], f32, tag="S", name=f"qkp{ct}_{t4}")
                ps = pst[:, 0:512]
                for c in range(4):
                    nc.tensor.matmul(
                        ps,
                        wqk_s[:, c, ct * 128:(ct + 1) * 128],
                        xT[:, c, t4 * 512:(t4 + 1) * 512],
                        start=(c == 0),
                        stop=(c == 3),
                    )
                nc.vector.tensor_scalar_mul(
                    qkT[:, ct, t4 * 512:(t4 + 1) * 512], ps, 1.0
                )

            def emit_v(tt):
                pst = psS2.tile([128, 1024], f32, tag="S", name=f"vp{tt}")
                psv = pst[:, 0:256]
                for c in range(4):
                    nc.tensor.matmul(
                        psv,
                        xT[:, c, tt * 128:(tt + 1) * 128],
                        wv_s[:, c, :],
                        start=(c == 0),
                        stop=(c == 3),
                    )
                nc.vector.tensor_scalar_mul(
                    vaug8[:, tt, :, 0:64],
                    psv.rearrange("p (h e) -> p h e", e=64),
                    1.0,
                )
                if tt < 4:
                    nc.vector.tensor_scalar_mul(
                        vaug16[:, tt, :, 0:64],
                        psv.rearrange("p (h e) -> p h e", e=64),
                        1.0,
                    )

            def emit_outproj(tt, drain=False):
                pst = psS2.tile([128, 1024], f32, tag="S", name=f"opp{tt}")
                po = pst[:, 0:512]
                nc.tensor.matmul(
                    po, yTn[:, 0, tt * 128:(tt + 1) * 128], wo_s[:, 0, :],
                    start=True, stop=False,
                )
                nc.tensor.matmul(
                    po, yTn[:, 1, tt * 128:(tt + 1) * 128], wo_s[:, 1, :],
                    start=False, stop=True,
                )
                ot = opool.tile([128, 512], f32, tag="o", name=f"ot{tt}")
                if drain and tt % 2 == 0:
                    # ACT is idle during the drain; parallelize the copies
                    nc.scalar.copy(ot[:], po)
                else:
                    nc.vector.tensor_scalar_mul(ot[:], po, 1.0)
                eng = nc.gpsimd if tt % 2 == 0 else nc.sync
                eng.dma_start(out_d.ap()[tt * 128:(tt + 1) * 128, :], ot[:])

            # attention S+exp unit; j < 2*it: non-diag chunk pair (DR),
            # else diag pair dpair = j - 2*it
            def emit_S_unit(it, h, j):
                po2 = (h % 2) * 64
                ct_q, ct_k = h // 2, 2 + h // 2
                i0 = it * 512
                if j < 2 * it:
                    jp = j
                    qrow = qkT[po2:po2 + 64, ct_q, i0:i0 + 512]
                    psS = psS2.tile(
                        [128, 2, 512], f32, tag="S", name=f"S{it}_{h}_{j}"
                    )
                    for u in (0, 1):
                        jc = 2 * jp + u
                        nc.tensor.matmul(
                            psS[:, u, :],
                            qkT[po2:po2 + 64, ct_k, jc * 128:(jc + 1) * 128],
                            qrow,
                            start=True,
                            stop=True,
                        )
                    pr = ppool.tile([128, 2, 512], f8p, tag="P8", name=f"pr{it}_{h}_{j}")
                    nc.scalar.activation(pr[:], psS[:], EXPF, scale=SCALE, bias=ebias[:])
                    return pr
                dpair = j - 2 * it
                s0 = 2 * dpair
                widths = [512 - 128 * s0, 512 - 128 * (s0 + 1)]
                tot = widths[0] + widths[1]
                psD = psS2.tile([128, 1024], f32, tag="S", name=f"D{it}_{h}_{j}")
                offs = [0, widths[0]]
                chunks = []
                for i, s in enumerate((s0, s0 + 1)):
                    jc = 4 * it + s
                    kap = qkT[po2:po2 + 64, ct_k, jc * 128:(jc + 1) * 128]
                    qs = qkT[po2:po2 + 64, ct_q, i0 + 128 * s:i0 + 512]
                    chunks.append((offs[i], widths[i], kap, qs))
                # One full-W S matmul + one W-wide causal-mask add per chunk
                # (eident is identity in its first 128 cols, zero beyond, so
                # the mask lands only on the leading 128x128 block).
                # start=True marks the whole 2KB psum BANK pending-zero, so
                # interleaving S/mask across chunks is only safe when the two
                # chunks sit in different banks (dpair 0: offsets 0/512).
                if dpair == 0:
                    for off, W, kap, qs in chunks:
                        nc.tensor.matmul(
                            psD[:, off:off + W], kap, qs, start=True, stop=False,
                        )
                    for off, W, kap, qs in chunks:
                        nc.tensor.matmul(
                            psD[:, off:off + W], uprime[:], e512[:, 0:W],
                            start=False, stop=True,
                        )
                else:
                    for off, W, kap, qs in chunks:
                        nc.tensor.matmul(
                            psD[:, off:off + W], kap, qs, start=True, stop=False,
                        )
                        nc.tensor.matmul(
                            psD[:, off:off + W], uprime[:], e512[:, 0:W],
                            start=False, stop=True,
                        )
                if it == 0:
                    prD = ppool.tile([128, tot], bf16, tag="PD16", name=f"pd{h}_{j}")
                else:
                    prD = ppool.tile([128, tot], f8p, tag="PD8", name=f"pd{it}_{h}_{j}")
                nc.scalar.activation(prD[:], psD[:, 0:tot], EXPF, scale=SCALE, bias=ebias[:])
                return prD

            def emit_PV_unit(it, h, pv, j, pr):
                first = (j == 0)
                if j < 2 * it:
                    jp = j
                    nc.tensor.matmul(
                        pv[:],
                        vaug8[:, 2 * jp:2 * jp + 2, h, :],
                        pr[:],
                        start=first,
                        stop=False,
                        perf_mode=DR,
                    )
                    return
                dpair = j - 2 * it
                s0 = 2 * dpair
                vsrc = vaug16 if it == 0 else vaug8
                off = 0
                for s in (s0, s0 + 1):
                    jc = 4 * it + s
                    W = 512 - 128 * s
                    vch = vsrc[:, s if it == 0 else jc, h, :]
                    nc.tensor.matmul(
                        pv[:, 128 * s:512],
                        vch,
                        pr[:, off:off + W],
                        start=(first and s == 0),
                        stop=(s == 3),
                    )
                    off += W

            def emit_normalize(it, h, pv):
                po2 = (h % 2) * 64
                i0 = it * 512
                # NB: custom-DVE recip cannot read PSUM at a partition offset;
                # copy l to SBUF first (regular ops handle the offset fine).
                ls = rpool.tile([64, 512], f32, tag="l", name=f"ls{it}_{h}")
                nc.vector.tensor_scalar_mul(ls[:], pv[64:128, :], 1.0)
                rT = rpool.tile([64, 512], f32, tag="r", name=f"rt{it}_{h}")
                nc.vector.reciprocal_approx_fast(rT[:], ls[:])
                nc.vector.tensor_mul(
                    yTn[po2:po2 + 64, h // 2, i0:i0 + 512], pv[0:64, :], rT[:]
                )

            # ---- the pipelined stream ----
            # startup: projections for t4=0 (attention i-tile 0 inputs).
            # Order so head-pair (0,1)'s S inputs (q ct0, k ct2) land first.
            emit_qk(0, 0)
            emit_qk(2, 0)
            for tt in range(4):
                emit_v(tt)
            emit_qk(1, 0)
            emit_qk(3, 0)

            def attention_pair(it, ha, hb, fillers):
                nunits = 2 * it + 2
                pva = psPV.tile([128, 512], f32, tag="PV", name=f"pva{it}_{ha}")
                pvb = psPV.tile([128, 512], f32, tag="PV", name=f"pvb{it}_{hb}")
                pra = emit_S_unit(it, ha, 0)
                prb = emit_S_unit(it, hb, 0)
                for j in range(nunits):
                    if j + 1 < nunits:
                        na = emit_S_unit(it, ha, j + 1)
                    if fillers:
                        fillers.pop(0)()
                    emit_PV_unit(it, ha, pva, j, pra)
                    if j + 1 < nunits:
                        nb = emit_S_unit(it, hb, j + 1)
                    if fillers:
                        fillers.pop(0)()
                    emit_PV_unit(it, hb, pvb, j, prb)
                    if j + 1 < nunits:
                        pra, prb = na, nb
                emit_normalize(it, ha, pva)
                emit_normalize(it, hb, pvb)

            from functools import partial

            for it in range(4):
                fillers = []
                if it < 3:
                    for ct in range(4):
                        fillers.append(partial(emit_qk, ct, it + 1))
                    for tt in range(4 * (it + 1), 4 * (it + 2)):
                        fillers.append(partial(emit_v, tt))
                if it >= 1:
                    for tt in range(4 * (it - 1), 4 * it):
                        fillers.append(partial(emit_outproj, tt))
                # split fillers across the two head pairs
                half = (len(fillers) + 1) // 2
                fa, fb = fillers[:half], fillers[half:]
                attention_pair(it, 0, 1, fa)
                attention_pair(it, 2, 3, fb)
                for f in fa + fb:  # any unconsumed filler slots
                    f()

            # drain: output projection for i-tile 3
            for tt in range(12, 16):
                emit_outproj(tt, drain=True)

            if DEBUG:
                nc.sync.dma_start(dbg["xT"].ap(), xT[:])
                nc.sync.dma_start(dbg["qkT"].ap(), qkT[:])
                nc.sync.dma_start(dbg["vaug"].ap(), vaug8[:])
                nc.sync.dma_start(dbg["yTn"].ap(), yTn[:])

    nc.compile()
    return nc


def _get_nc():
    global _BUILT
    if _BUILT is None:
        _BUILT = _build()
    return _BUILT


def _make_in_maps(x, Wqkv, Wout):
    q, k, v = Wqkv[:, 0:512], Wqkv[:, 512:1024], Wqkv[:, 1024:1536]
    b16 = ml_dtypes.bfloat16
    in_maps = []
    for core in range(NCORES):
        b, g = core // 2, core % 2
        hs = [g * 4 + i for i in range(4)]
        wqk = np.concatenate(
            [q[:, h * 64:(h + 1) * 64] for h in hs]
            + [k[:, h * 64:(h + 1) * 64] for h in hs],
            axis=1,
        )  # [512, 512]
        # rows (c p) -> p c m
        wqk = np.ascontiguousarray(
            wqk.reshape(4, 128, 512).transpose(1, 0, 2)
        ).astype(b16)
        wv = np.ascontiguousarray(
            v[:, g * 256:(g + 1) * 256].reshape(4, 128, 256).transpose(1, 0, 2)
        ).astype(b16)
        wo = np.ascontiguousarray(
            Wout[g * 256:(g + 1) * 256, :].reshape(2, 128, 512).transpose(1, 0, 2)
        ).astype(b16)
        in_maps.append(
            {
                "x": np.ascontiguousarray(x[b]).astype(b16),
                "wqk": wqk,
                "wv": wv,
                "wo": wo,
            }
        )
    return in_maps


def _run(x, Wqkv, Wout, trace=False):
    nc = _get_nc()
    in_maps = _make_in_maps(x, Wqkv, Wout)
    res = run_bass_kernel_spmd(
        nc, in_maps, core_ids=list(range(NCORES)), trace=trace
    )
    out = np.empty((B, T, D), dtype=np.float32)
    for b in range(B):
        out[b] = res.results[2 * b]["out"] + res.results[2 * b + 1]["out"]
    return out, res


def _reference_fallback(x, attn_mask, Wqkv, Wout):
    # general (non-causal-mask) path: plain numpy
    qkv = x @ Wqkv
    q, k, v = np.split(qkv, 3, axis=-1)

    def heads(t):
        return t.reshape(B, T, H, HD).transpose(0, 2, 1, 3)

    q, k, v = heads(q), heads(k), heads(v)
    att = np.einsum("bhqd,bhkd->bhqk", q, k) * SCALE
    att = np.where(attn_mask[None, None] == 0, -np.inf, att)
    att = att - att.max(axis=-1, keepdims=True)
    att = np.exp(att)
    att = att / att.sum(axis=-1, keepdims=True)
    y = np.einsum("bhqk,bhkd->bhqd", att, v)
    return (y.transpose(0, 2, 1, 3).reshape(B, T, D) @ Wout).astype(np.float32)


def kernel(x, attn_mask, Wqkv, Wout):
    x = np.asarray(x, dtype=np.float32)
    attn_mask = np.asarray(attn_mask)
    Wqkv = np.asarray(Wqkv, dtype=np.float32)
    Wout = np.asarray(Wout, dtype=np.float32)

    causal = bool(
        np.array_equal(attn_mask != 0, np.tril(np.ones((T, T), dtype=bool)))
    )
    if not causal:
        return _reference_fallback(x, attn_mask, Wqkv, Wout)

    out, _ = _run(x, Wqkv, Wout, trace=False)
    return out


# revision 33
# speedup vs baseline: 1.2124x; 1.2124x over previous
"""Multi-head self-attention Trainium2 kernel (B=4, T=2048, D=512, H=8, HD=64).

Sharding: 8 cores = 4 batches x 2 head-groups (4 heads each). Each core:
  - DMA-XBAR-transposes bf16 x into SBUF (no PE transposes),
  - projects q,k (transposed layout) and v (natural layout, with 64 ones
    columns per head so the softmax denominator comes out of the PV matmul
    pre-broadcast across 64 partitions),
  - causal flash attention in transposed space, all matmuls bf16 except the
    PV step on i-tiles 1-3 which runs fp8 DoubleRow (two 128-key chunks
    contracted per pass at 0.5 cycles/row; v+ones in fp8e4, probs in fp8e5
    whose wide exponent range avoids inf/subnormal cliffs). i-tile 0 (rows
    with few valid keys, where fp8 noise does not average out) stays bf16.
  - exp on ACT with scale=1/8 and bias=-2 folded in (bias cancels in the
    softmax normalization; keeps probs inside fp8e5 range),
  - normalizes with reciprocal_approx_fast + tensor_mul,
  - output projection for its 256 contraction rows -> partial [2048, 512].
Host sums the 2 partials per batch (the TP all-reduce of the hint).

Scheduling: the whole kernel is emitted as one software-pipelined stream.
Attention processes two heads at once (ping-pong) so the PE computes one
head's S scores while the ACT engine exponentiates the other's, and
projection / output-projection matmul groups are injected as filler work
into the remaining exp-wait slots to keep the PE dense (sustains the 2.4GHz
p-state). PSUM: one shared 3-deep ring of 2-bank tiles (S scores + all
projections) + 2 PV accumulators = 8 banks.
"""

import sys

sys.path.insert(0, "/opt/trn_rl_repo")

import numpy as np
import ml_dtypes

import concourse.bass as bass
import concourse.tile as tile
from concourse import bacc, masks, mybir
from concourse.bass_utils import run_bass_kernel_spmd

f32 = mybir.dt.float32
bf16 = mybir.dt.bfloat16
f8 = mybir.dt.float8e4
f8p = mybir.dt.float8e5
u32 = mybir.dt.uint32

B, T, D, H, HD = 4, 2048, 512, 8, 64
NCORES = 8
SCALE = 1.0 / np.sqrt(HD)  # 0.125
EXP_BIAS = -2.0
NEG = -1.0e30
EXPF = mybir.ActivationFunctionType.Exp
DR = mybir.MatmulPerfMode.DoubleRow

_BUILT = None
DEBUG = False


def _build():
    nc = bacc.Bacc("TRN2", target_bir_lowering=False, debug=False)

    x_d = nc.dram_tensor("x", [T, D], bf16, kind="ExternalInput")
    wqk_d = nc.dram_tensor("wqk", [128, 4, 512], bf16, kind="ExternalInput")
    wv_d = nc.dram_tensor("wv", [128, 4, 256], bf16, kind="ExternalInput")
    wo_d = nc.dram_tensor("wo", [128, 2, 512], bf16, kind="ExternalInput")
    out_d = nc.dram_tensor("out", [T, D], f32, kind="ExternalOutput")
    dbg = {}
    if DEBUG:
        dbg["xT"] = nc.dram_tensor("dbg_xT", [128, 4, T], bf16, kind="ExternalOutput")
        dbg["qkT"] = nc.dram_tensor("dbg_qkT", [128, 4, T], bf16, kind="ExternalOutput")
        dbg["vaug"] = nc.dram_tensor(
            "dbg_vaug", [128, 16, 4, 128], f8, kind="ExternalOutput"
        )
        dbg["yTn"] = nc.dram_tensor("dbg_yTn", [128, 2, T], bf16, kind="ExternalOutput")

    with tile.TileContext(nc) as tc:
        with (
            tc.tile_pool(name="big", bufs=1) as big,
            tc.tile_pool(name="pp", bufs=5) as ppool,
            tc.tile_pool(name="rp", bufs=3) as rpool,
            tc.tile_pool(name="op", bufs=3) as opool,
            tc.tile_pool(name="ps2", bufs=3, space=bass.MemorySpace.PSUM) as psS2,
            tc.tile_pool(name="pv", bufs=2, space=bass.MemorySpace.PSUM) as psPV,
        ):
            # ---- persistent sbuf tensors ----
            xT = big.tile([128, 4, T], bf16)     # [d_part, d_chunk, t]
            qkT = big.tile([128, 4, T], bf16)    # ct: 0=q(h0,h1) 1=q(h2,h3) 2=k(h0,h1) 3=k(h2,h3)
            vaug8 = big.tile([128, 16, 4, 128], f8)   # [t_part, t_tile, head, 64 v + 64 ones]
            vaug16 = big.tile([128, 4, 4, 128], bf16)  # chunks 0-3 only (i-tile 0)
            yTn = big.tile([128, 2, T], bf16)    # [c_part, c_chunk, t]
            wqk_s = big.tile([128, 4, 512], bf16)
            wv_s = big.tile([128, 4, 256], bf16)
            wo_s = big.tile([128, 2, 512], bf16)

            # ---- weight loads (gpsimd SWDGE queue; hwdge queues do x) ----
            nc.sync.dma_start(wqk_s[:], wqk_d.ap())
            nc.sync.dma_start(wv_s[:], wv_d.ap())
            nc.sync.dma_start(wo_s[:], wo_d.ap())

            # ---- constants ----
            up_f = big.tile([128, 128], f32)
            masks.make_causal_mask(nc, up_f[:], mask_val=NEG)  # [k,m]=NEG where m>k
            uprime = big.tile([128, 128], bf16)
            nc.vector.tensor_scalar_mul(uprime[:], up_f[:], 1.0)
            eid_f = big.tile([128, 512], f32)
            nc.vector.memset(eid_f[:], 0.0)
            masks.make_identity(nc, eid_f[:, 0:128], nomemset=True)
            e512 = big.tile([128, 512], bf16)
            nc.vector.tensor_scalar_mul(e512[:], eid_f[:], 1.0)

            # ones columns: fp8e4(1.0)=0x38, bf16(1.0)=0x3F80
            nc.vector.memset(vaug8[:, :, :, 64:128].bitcast(u32), 0x38383838)
            nc.vector.memset(vaug16[:, :, :, 64:128].bitcast(u32), 0x3F803F80)

            ebias = big.tile([128, 1], f32)
            nc.vector.memset(ebias[:], EXP_BIAS)

            # ---- phase 1: XBAR dma transposes of x into xT, t4-chunked,
            # split across both HWDGE queues (sync + scalar) ----
            # NB: XBAR transposes must all stay on ONE queue — two queues
            # driving the XBAR concurrently corrupt each other's tiles.
            for t4 in range(4):
                for cc in range(4):
                    eng = nc.sync
                    eng.dma_start_transpose(
                        xT[:, cc, t4 * 512:(t4 + 1) * 512],
                        x_d.ap()[t4 * 512:(t4 + 1) * 512, cc * 128:(cc + 1) * 128],
                    )

            # ---- emission helpers (each is a self-contained PE work unit) --
            def emit_qk(ct, t4):
                pst = psS2.tile([128, 1024], f32, tag="S", name=f"qkp{ct}_{t4}")
                ps = pst[:, 0:512]
                for c in range(4):
                    nc.tensor.matmul(
                        ps,
                        wqk_s[:, c, ct * 128:(ct + 1) * 128],
                        xT[:, c, t4 * 512:(t4 + 1) * 512],
                        start=(c == 0),
                        stop=(c == 3),
                    )
                nc.vector.tensor_scalar_mul(
                    qkT[:, ct, t4 * 512:(t4 + 1) * 512], ps, 1.0
                )

            def emit_v(tt):
                pst = psS2.tile([128, 1024], f32, tag="S", name=f"vp{tt}")
                psv = pst[:, 0:256]
                for c in range(4):
                    nc.tensor.matmul(
                        psv,
                        xT[:, c, tt * 128:(tt + 1) * 128],
                        wv_s[:, c, :],
                        start=(c == 0),
                        stop=(c == 3),
                    )
                nc.vector.tensor_scalar_mul(
                    vaug8[:, tt, :, 0:64],
                    psv.rearrange("p (h e) -> p h e", e=64),
                    1.0,
                )
                if tt < 4:
                    nc.vector.tensor_scalar_mul(
                        vaug16[:, tt, :, 0:64],
                        psv.rearrange("p (h e) -> p h e", e=64),
                        1.0,
                    )

            def emit_outproj(tt, drain=False):
                pst = psS2.tile([128, 1024], f32, tag="S", name=f"opp{tt}")
                po = pst[:, 0:512]
                nc.tensor.matmul(
                    po, yTn[:, 0, tt * 128:(tt + 1) * 128], wo_s[:, 0, :],
                    start=True, stop=False,
                )
                nc.tensor.matmul(
                    po, yTn[:, 1, tt * 128:(tt + 1) * 128], wo_s[:, 1, :],
                    start=False, stop=True,
                )
                ot = opool.tile([128, 512], f32, tag="o", name=f"ot{tt}")
                if drain and tt % 2 == 0:
                    # ACT is idle during the drain; parallelize the copies
                    nc.scalar.copy(ot[:], po)
                else:
                    nc.vector.tensor_scalar_mul(ot[:], po, 1.0)
                eng = nc.gpsimd if tt % 2 == 0 else nc.sync
                eng.dma_start(out_d.ap()[tt * 128:(tt + 1) * 128, :], ot[:])

            # attention S+exp unit; j < 2*it: non-diag chunk pair (DR),
            # else diag pair dpair = j - 2*it
            def emit_S_unit(it, h, j):
                po2 = (h % 2) * 64
                ct_q, ct_k = h // 2, 2 + h // 2
                i0 = it * 512
                if j < 2 * it:
                    jp = j
                    qrow = qkT[po2:po2 + 64, ct_q, i0:i0 + 512]
                    psS = psS2.tile(
                        [128, 2, 512], f32, tag="S", name=f"S{it}_{h}_{j}"
                    )
                    for u in (0, 1):
                        jc = 2 * jp + u
                        nc.tensor.matmul(
                            psS[:, u, :],
                            qkT[po2:po2 + 64, ct_k, jc * 128:(jc + 1) * 128],
                            qrow,
                            start=True,
                            stop=True,
                        )
                    pr = ppool.tile([128, 2, 512], f8p, tag="P8", name=f"pr{it}_{h}_{j}")
                    nc.scalar.activation(pr[:], psS[:], EXPF, scale=SCALE, bias=ebias[:])
                    return pr
                dpair = j - 2 * it
                s0 = 2 * dpair
                widths = [512 - 128 * s0, 512 - 128 * (s0 + 1)]
                tot = widths[0] + widths[1]
                psD = psS2.tile([128, 1024], f32, tag="S", name=f"D{it}_{h}_{j}")
                offs = [0, widths[0]]
                chunks = []
                for i, s in enumerate((s0, s0 + 1)):
                    jc = 4 * it + s
                    kap = qkT[po2:po2 + 64, ct_k, jc * 128:(jc + 1) * 128]
                    qs = qkT[po2:po2 + 64, ct_q, i0 + 128 * s:i0 + 512]
                    chunks.append((offs[i], widths[i], kap, qs))
                # One full-W S matmul + one W-wide causal-mask add per chunk
                # (eident is identity in its first 128 cols, zero beyond, so
                # the mask lands only on the leading 128x128 block).
                # start=True marks the whole 2KB psum BANK pending-zero, so
                # interleaving S/mask across chunks is only safe when the two
                # chunks sit in different banks (dpair 0: offsets 0/512).
                if dpair == 0:
                    for off, W, kap, qs in chunks:
                        nc.tensor.matmul(
                            psD[:, off:off + W], kap, qs, start=True, stop=False,
                        )
                    for off, W, kap, qs in chunks:
                        nc.tensor.matmul(
                            psD[:, off:off + W], uprime[:], e512[:, 0:W],
                            start=False, stop=True,
                        )
                else:
                    for off, W, kap, qs in chunks:
                        nc.tensor.matmul(
                            psD[:, off:off + W], kap, qs, start=True, stop=False,
                        )
                        nc.tensor.matmul(
                            psD[:, off:off + W], uprime[:], e512[:, 0:W],
                            start=False, stop=True,
                        )
                if it == 0:
                    prD = ppool.tile([128, tot], bf16, tag="PD16", name=f"pd{h}_{j}")
                else:
                    prD = ppool.tile([128, tot], f8p, tag="PD8", name=f"pd{it}_{h}_{j}")
                nc.scalar.activation(prD[:], psD[:, 0:tot], EXPF, scale=SCALE, bias=ebias[:])
                return prD

            def emit_PV_unit(it, h, pv, j, pr):
                first = (j == 0)
                if j < 2 * it:
                    jp = j
                    nc.tensor.matmul(
                        pv[:],
                        vaug8[:, 2 * jp:2 * jp + 2, h, :],
                        pr[:],
                        start=first,
                        stop=False,
                        perf_mode=DR,
                    )
                    return
                dpair = j - 2 * it
                s0 = 2 * dpair
                vsrc = vaug16 if it == 0 else vaug8
                off = 0
                for s in (s0, s0 + 1):
                    jc = 4 * it + s
                    W = 512 - 128 * s
                    vch = vsrc[:, s if it == 0 else jc, h, :]
                    nc.tensor.matmul(
                        pv[:, 128 * s:512],
                        vch,
                        pr[:, off:off + W],
                        start=(first and s == 0),
                        stop=(s == 3),
                    )
                    off += W

            def emit_normalize(it, h, pv):
                po2 = (h % 2) * 64
                i0 = it * 512
                # NB: custom-DVE recip cannot read PSUM at a partition offset;
                # copy l to SBUF first (regular ops handle the offset fine).
                ls = rpool.tile([64, 512], f32, tag="l", name=f"ls{it}_{h}")
                nc.vector.tensor_scalar_mul(ls[:], pv[64:128, :], 1.0)
                rT = rpool.tile([64, 512], f32, tag="r", name=f"rt{it}_{h}")
                nc.vector.reciprocal_approx_fast(rT[:], ls[:])
                nc.vector.tensor_mul(
                    yTn[po2:po2 + 64, h // 2, i0:i0 + 512], pv[0:64, :], rT[:]
                )

            # ---- the pipelined stream ----
            # startup: projections for t4=0 (attention i-tile 0 inputs).
            # Order so head-pair (0,1)'s S inputs (q ct0, k ct2) land first.
            emit_qk(0, 0)
            emit_qk(2, 0)
            for tt in range(4):
                emit_v(tt)
            emit_qk(1, 0)
            emit_qk(3, 0)

            def attention_pair(it, ha, hb, fillers):
                nunits = 2 * it + 2
                pva = psPV.tile([128, 512], f32, tag="PV", name=f"pva{it}_{ha}")
                pvb = psPV.tile([128, 512], f32, tag="PV", name=f"pvb{it}_{hb}")
                pra = emit_S_unit(it, ha, 0)
                prb = emit_S_unit(it, hb, 0)
                for j in range(nunits):
                    if j + 1 < nunits:
                        na = emit_S_unit(it, ha, j + 1)
                    if fillers:
                        fillers.pop(0)()
                    emit_PV_unit(it, ha, pva, j, pra)
                    if j + 1 < nunits:
                        nb = emit_S_unit(it, hb, j + 1)
                    if fillers:
                        fillers.pop(0)()
                    emit_PV_unit(it, hb, pvb, j, prb)
                    if j + 1 < nunits:
                        pra, prb = na, nb
                emit_normalize(it, ha, pva)
                emit_normalize(it, hb, pvb)

            from functools import partial

            for it in range(4):
                fillers = []
                if it < 3:
                    for ct in range(4):
                        fillers.append(partial(emit_qk, ct, it + 1))
                    for tt in range(4 * (it + 1), 4 * (it + 2)):
                        fillers.append(partial(emit_v, tt))
                if it >= 1:
                    for tt in range(4 * (it - 1), 4 * it):
                        fillers.append(partial(emit_outproj, tt))
                # split fillers across the two head pairs
                half = (len(fillers) + 1) // 2
                fa, fb = fillers[:half], fillers[half:]
                attention_pair(it, 0, 1, fa)
                attention_pair(it, 2, 3, fb)
                for f in fa + fb:  # any unconsumed filler slots
                    f()

            # drain: output projection for i-tile 3
            for tt in range(12, 16):
                emit_outproj(tt, drain=True)

            if DEBUG:
                nc.sync.dma_start(dbg["xT"].ap(), xT[:])
                nc.sync.dma_start(dbg["qkT"].ap(), qkT[:])
                nc.sync.dma_start(dbg["vaug"].ap(), vaug8[:])
                nc.sync.dma_start(dbg["yTn"].ap(), yTn[:])

    nc.compile()
    return nc


def _get_nc():
    global _BUILT
    if _BUILT is None:
        _BUILT = _build()
    return _BUILT


def _make_in_maps(x, Wqkv, Wout):
    q, k, v = Wqkv[:, 0:512], Wqkv[:, 512:1024], Wqkv[:, 1024:1536]
    b16 = ml_dtypes.bfloat16
    in_maps = []
    for core in range(NCORES):
        b, g = core // 2, core % 2
        hs = [g * 4 + i for i in range(4)]
        wqk = np.concatenate(
            [q[:, h * 64:(h + 1) * 64] for h in hs]
            + [k[:, h * 64:(h + 1) * 64] for h in hs],
            axis=1,
        )  # [512, 512]
        # rows (c p) -> p c m
        wqk = np.ascontiguousarray(
            wqk.reshape(4, 128, 512).transpose(1, 0, 2)
        ).astype(b16)
        wv = np.ascontiguousarray(
            v[:, g * 256:(g + 1) * 256].reshape(4, 128, 256).transpose(1, 0, 2)
        ).astype(b16)
        wo = np.ascontiguousarray(
            Wout[g * 256:(g + 1) * 256, :].reshape(2, 128, 512).transpose(1, 0, 2)
        ).astype(b16)
        in_maps.append(
            {
                "x": np.ascontiguousarray(x[b]).astype(b16),
                "wqk": wqk,
                "wv": wv,
                "wo": wo,
            }
        )
    return in_maps


def _run(x, Wqkv, Wout, trace=False):
    nc = _get_nc()
    in_maps = _make_in_maps(x, Wqkv, Wout)
    res = run_bass_kernel_spmd(
        nc, in_maps, core_ids=list(range(NCORES)), trace=trace
    )
    out = np.empty((B, T, D), dtype=np.float32)
    for b in range(B):
        out[b] = res.results[2 * b]["out"] + res.results[2 * b + 1]["out"]
    return out, res


def _reference_fallback(x, attn_mask, Wqkv, Wout):
    # general (non-causal-mask) path: plain numpy
    qkv = x @ Wqkv
    q, k, v = np.split(qkv, 3, axis=-1)

    def heads(t):
        return t.reshape(B, T, H, HD).transpose(0, 2, 1, 3)

    q, k, v = heads(q), heads(k), heads(v)
    att = np.einsum("bhqd,bhkd->bhqk", q, k) * SCALE
    att = np.where(attn_mask[None, None] == 0, -np.inf, att)
    att = att - att.max(axis=-1, keepdims=True)
    att = np.exp(att)
    att = att / att.sum(axis=-1, keepdims=True)
    y = np.einsum("bhqk,bhkd->bhqd", att, v)
    return (y.transpose(0, 2, 1, 3).reshape(B, T, D) @ Wout).astype(np.float32)


def kernel(x, attn_mask, Wqkv, Wout):
    x = np.asarray(x, dtype=np.float32)
    attn_mask = np.asarray(attn_mask)
    Wqkv = np.asarray(Wqkv, dtype=np.float32)
    Wout = np.asarray(Wout, dtype=np.float32)

    causal = bool(
        np.array_equal(attn_mask != 0, np.tril(np.ones((T, T), dtype=bool)))
    )
    if not causal:
        return _reference_fallback(x, attn_mask, Wqkv, Wout)

    out, _ = _run(x, Wqkv, Wout, trace=False)
    return out
